# revision 1
# baseline (speedup 1.0000x reference)
"""MOLVAE forward pass, data-parallel over graphs across 8 NeuronCores.

Strategy: shard the 2048 graphs into 8 contiguous blocks of 256 graphs
(8192 nodes, 32768 edges, 126976 decoder pairs per core). Graphs are
independent for message passing and the triu decode; only the BatchNorm
statistics couple shards. We run the network in stages on-device and
combine the tiny per-shard BN sums on the host between stages, folding
the normalization into a per-feature scale/shift applied at the start of
the next stage. Edges arrive grouped by destination (4 incoming edges
per node), which lets the scatter-softmax become a dense reshape —
no segment_* scatter ops on device, only row gathers k[src]/v[src].
"""

import math
import numpy as np
import jax
import jax.numpy as jnp

HEADS = 4
ENC = 64
LAT = 64
EDGE_DIM = 11
DEC = 128
NUM_ENC_LAYERS = 4
NUM_DEC_HIDDEN = 2
N_GRAPHS = 2048
NPG = 32                      # nodes per graph
N_NODES = N_GRAPHS * NPG
DEG = 4
N_EDGES = N_NODES * DEG
M = 8                         # cores
GPS = N_GRAPHS // M           # graphs per shard
NPS = GPS * NPG               # nodes per shard
EPS_E = NPS * DEG             # edges per shard
NPAIR = NPG * (NPG - 1) // 2  # 496 pairs per graph
BN_EPS = 1e-5

_IU, _JU = np.triu_indices(NPG, k=1)


def _tconv(x, src_local, edge_attr, p, dout):
    """PyG TransformerConv (heads=4, concat=False, beta=True, edge_dim set),
    specialized to exactly DEG incoming edges per node, edge rows grouped by
    destination node."""
    N = x.shape[0]
    q = (x @ p['Wq'] + p['bq']).reshape(N, 1, HEADS, dout)
    k = (x @ p['Wk'] + p['bk']).reshape(N, HEADS, dout)
    v = (x @ p['Wv'] + p['bv']).reshape(N, HEADS, dout)
    e = (edge_attr @ p['We']).reshape(N, DEG, HEADS, dout)
    ks = k[src_local].reshape(N, DEG, HEADS, dout)
    vs = v[src_local].reshape(N, DEG, HEADS, dout)
    alpha = (q * (ks + e)).sum(-1) / math.sqrt(dout)      # [N, DEG, H]
    amax = alpha.max(axis=1, keepdims=True)
    ex = jnp.exp(alpha - amax)
    w = ex / ex.sum(axis=1, keepdims=True)                # softmax over DEG
    out = ((vs + e) * w[..., None]).sum(axis=1).mean(axis=1)
    x_r = x @ p['Wskip'] + p['bskip']
    b = jax.nn.sigmoid(jnp.concatenate([out, x_r, out - x_r], -1) @ p['Wbeta'])
    return b * x_r + (1.0 - b) * out


def _stats(h):
    return h.sum(0), (h * h).sum(0)


# ---- staged device functions (mapped across the 8 cores) ----

def _enc_first(x, src, ea, conv):
    hp = _tconv(x, src, ea, conv, ENC)
    s, ss = _stats(hp)
    return hp, s, ss


def _enc_mid(hp, scale, shift, src, ea, conv):
    h = jax.nn.relu(hp * scale + shift)
    hp2 = _tconv(h, src, ea, conv, ENC)
    s, ss = _stats(hp2)
    return hp2, s, ss


def _latent(hp, scale, shift, src, ea, eps, mu_p, lv_p, W1, b1):
    h = jax.nn.relu(hp * scale + shift)
    mu = _tconv(h, src, ea, mu_p, LAT)
    lv = _tconv(h, src, ea, lv_p, LAT)
    z = mu + eps * jnp.exp(0.5 * lv)
    zg = z.reshape(GPS, NPG, LAT)
    feats = jnp.concatenate([zg[:, _IU, :], zg[:, _JU, :]], -1)
    feats = feats.reshape(GPS * NPAIR, 2 * LAT)
    d1 = feats @ W1 + b1
    s, ss = _stats(d1)
    return mu, lv, d1, s, ss


def _dec_mid(d1, scale, shift, W2, b2):
    h = jax.nn.relu(d1 * scale + shift)
    d2 = h @ W2 + b2
    s, ss = _stats(d2)
    return d2, s, ss


def _dec_out(d2, scale, shift, Wo, bo):
    h = jax.nn.relu(d2 * scale + shift)
    return h @ Wo + bo


_PMAPPED = {}


def _get_pmapped():
    if _PMAPPED:
        return _PMAPPED
    _PMAPPED['enc_first'] = jax.pmap(_enc_first, in_axes=(0, 0, 0, None))
    _PMAPPED['enc_mid'] = jax.pmap(_enc_mid, in_axes=(0, None, None, 0, 0, None))
    _PMAPPED['latent'] = jax.pmap(
        _latent, in_axes=(0, None, None, 0, 0, 0, None, None, None, None))
    _PMAPPED['dec_mid'] = jax.pmap(_dec_mid, in_axes=(0, None, None, None, None))
    _PMAPPED['dec_out'] = jax.pmap(_dec_out, in_axes=(0, None, None, None, None))
    return _PMAPPED


def _fold_bn(s_shards, ss_shards, gamma, beta, n):
    """Combine per-shard sums into the affine scale/shift of the batchnorm."""
    s = np.asarray(s_shards, np.float64).sum(0)
    ss = np.asarray(ss_shards, np.float64).sum(0)
    m = s / n
    v = ss / n - m * m
    scale = np.asarray(gamma, np.float64) / np.sqrt(v + BN_EPS)
    shift = np.asarray(beta, np.float64) - m * scale
    return jnp.asarray(scale, jnp.float32), jnp.asarray(shift, jnp.float32)


def _np32(t):
    return np.asarray(t, np.float32)


def kernel(x, edge_attr, edge_index, batch_index, eps, params, num_graphs):
    fns = _get_pmapped()

    x = _np32(x).reshape(M, NPS, ENC)
    ea = _np32(edge_attr).reshape(M, EPS_E, EDGE_DIM)
    ep = _np32(eps).reshape(M, NPS, LAT)
    src = np.asarray(edge_index)[0].astype(np.int64).reshape(M, EPS_E)
    src = (src - (np.arange(M, dtype=np.int64) * NPS)[:, None]).astype(np.int32)

    tree32 = lambda t: jax.tree.map(_np32, t)
    enc = [tree32(lp) for lp in params['enc']]
    mu_p = tree32(params['mu'])
    lv_p = tree32(params['logvar'])
    dech = [tree32(dp) for dp in params['dec_hidden']]
    Wo = _np32(params['dec_out']['W'])
    bo = _np32(params['dec_out']['b'])

    # encoder layer 0
    hp, s, ss = fns['enc_first'](x, src, ea, enc[0]['conv'])
    scale, shift = _fold_bn(s, ss, enc[0]['gamma'], enc[0]['beta'], N_NODES)
    # encoder layers 1..3
    for li in range(1, NUM_ENC_LAYERS):
        hp, s, ss = fns['enc_mid'](hp, scale, shift, src, ea, enc[li]['conv'])
        scale, shift = _fold_bn(s, ss, enc[li]['gamma'], enc[li]['beta'], N_NODES)
    # latent + decoder layer 0 (pre-BN)
    mu, lv, d1, s, ss = fns['latent'](hp, scale, shift, src, ea, ep,
                                      mu_p, lv_p, dech[0]['W'], dech[0]['b'])
    npair_tot = N_GRAPHS * NPAIR
    scale, shift = _fold_bn(s, ss, dech[0]['gamma'], dech[0]['beta'], npair_tot)
    # decoder hidden layer 1 (pre-BN)
    d2, s, ss = fns['dec_mid'](d1, scale, shift, dech[1]['W'], dech[1]['b'])
    scale, shift = _fold_bn(s, ss, dech[1]['gamma'], dech[1]['beta'], npair_tot)
    # output
    logits = fns['dec_out'](d2, scale, shift, Wo, bo)

    logits = np.asarray(logits, np.float32).reshape(N_GRAPHS * NPAIR, 1)
    mu = np.asarray(mu, np.float32).reshape(N_NODES, LAT)
    lv = np.asarray(lv, np.float32).reshape(N_NODES, LAT)
    return logits, mu, lv
